# revision 25
# baseline (speedup 1.0000x reference)
"""Trainium2 Bass kernel for MultiHeadSelfAttention with relative position bias.

Sharding: 8 cores = 2 batches x 4 head-groups (4 heads each).
Each core computes LN -> QKV (its heads) -> scores+softmax+AV -> partial
out_proj; host sums the 4 partials per batch (the all-reduce) and adds b_out.

Key tricks:
1. rel_bias[i-j] is exactly rank-64 in (i, j) (sinusoid angle-addition), so
   it folds into the scores matmul as 64 extra contraction rows:
       scores+bias = [kT; Kb]^T @ [qT_scaled; Qb]   (K = 128, full PE array)
   No Toeplitz gather, no bias-add pass.
2. Softmax denominators come free via an all-ones column appended to V
   (row 64 of the AV output = sum_j exp); no max-subtraction is needed
   (|scores| < ~20, exp is safe in fp32) and the 1/sum normalization is
   deferred across the linear AV matmul into the OT evacuation.
3. float32r matmuls run at full PE rate (1 cycle/row at N>=512).
"""

import functools
import math
import sys

import numpy as np

for _p in ("/opt/trn_rl_repo", "/root/.axon_site/_ro/trn_rl_repo"):
    if _p not in sys.path:
        sys.path.insert(0, _p)

B, T, D, H, HD = 2, 2048, 1024, 16, 64
NCORES = 8
HPC = 4  # heads per core
NT = T // 128  # 16 row tiles
NI = T // 512  # 4 i-chunks (also LN i-groups of 4 row tiles)
F32R = True  # float32r (full-rate) matmuls; False = exact fp32 at 1/4 rate


@functools.lru_cache(maxsize=1)
def _build_nc():
    import concourse.mybir as mybir
    import concourse.tile as tile
    from concourse import bacc

    dt = mybir.dt
    f32 = dt.float32
    f32r = dt.float32r if F32R else f32
    AF = mybir.ActivationFunctionType
    ALU = mybir.AluOpType

    nc = bacc.Bacc("TRN2", target_bir_lowering=False, debug=False,
                   num_devices=NCORES)
    x = nc.declare_dram_parameter("x", [T, D], f32, isOutput=False)
    wq = nc.declare_dram_parameter("wq", [128, 8, 256], f32r, isOutput=False)
    wk = nc.declare_dram_parameter("wk", [128, 8, 256], f32r, isOutput=False)
    wv = nc.declare_dram_parameter("wv", [128, 8, 260], f32r, isOutput=False)
    bq = nc.declare_dram_parameter("bq", [1, 256], f32r, isOutput=False)
    bv = nc.declare_dram_parameter("bv", [1, 260], f32r, isOutput=False)
    qb = nc.declare_dram_parameter("qb", [HPC, 64, T], f32r, isOutput=False)
    kb = nc.declare_dram_parameter("kb", [64, T], f32r, isOutput=False)
    wo = nc.declare_dram_parameter("wo", [128, 2, D], f32r, isOutput=False)
    idin = nc.declare_dram_parameter("ident", [128, 128], f32, isOutput=False)
    onesd = nc.declare_dram_parameter("ones", [1, 512], f32r, isOutput=False)
    y = nc.declare_dram_parameter("y", [T, D], f32, isOutput=True)

    with tile.TileContext(nc) as tc:
        with tc.tile_pool(name="persist", bufs=1) as pp, \
             tc.tile_pool(name="big_ps", bufs=2, space="PSUM") as bigp, \
             tc.tile_pool(name="ot_ps", bufs=2, space="PSUM") as otpp:
            zero_col = pp.tile([128, 1], f32, tag="zero")
            nc.vector.memset(zero_col, 0.0)
            qhat = [pp.tile([128, T], f32r, tag=f"qhat{h}", name=f"qhat{h}")
                    for h in range(HPC)]
            khat = [pp.tile([128, T], f32r, tag=f"khat{h}", name=f"khat{h}")
                    for h in range(HPC)]
            v_sb = [pp.tile([128, 260], f32r, tag=f"v{jt}", name=f"v{jt}")
                    for jt in range(NT)]
            ot_pair = [pp.tile([128, T], f32r, tag=f"ot{p}", name=f"otp{p}")
                       for p in range(2)]
            wq_sb = pp.tile([128, 8, 256], f32r, tag="wq")
            wk_sb = pp.tile([128, 8, 256], f32r, tag="wk")
            wv_sb = pp.tile([128, 8, 260], f32r, tag="wv")
            wo_sb = pp.tile([128, 2, D], f32r, tag="wo")

            # ------------- Phase A+B interleaved per i-group -------------
            with tc.tile_pool(name="ab_sb", bufs=1) as abp, \
                 tc.tile_pool(name="ln", bufs=8) as lnp, \
                 tc.tile_pool(name="stats", bufs=4) as stp, \
                 tc.tile_pool(name="xnT", bufs=2) as xtp:
                ident = abp.tile([128, 128], f32, tag="ident")
                nc.sync.dma_start(out=ident, in_=idin[:, :])
                eps_t = abp.tile([128, 1], f32, tag="eps")
                nc.vector.memset(eps_t, 1e-5)
                ones_row = abp.tile([1, 512], f32r, tag="ones")
                nc.sync.dma_start(out=ones_row, in_=onesd[:, :])
                bq_row = abp.tile([1, 256], f32r, tag="bqr")
                nc.sync.dma_start(out=bq_row, in_=bq[:, :])
                bv_row = abp.tile([1, 260], f32r, tag="bvr")
                nc.sync.dma_start(out=bv_row, in_=bv[:, :])

                for ig in range(NI):
                    isl = slice(ig * 512, (ig + 1) * 512)
                    xts = []
                    for r in range(4):
                        it = ig * 4 + r
                        x_t = lnp.tile([128, D], f32, tag="x", name=f"x{it}")
                        nc.sync.dma_start(
                            out=x_t, in_=x[it * 128:(it + 1) * 128, :])
                        stats = stp.tile([128, 2, 6], f32, tag="st")
                        for sg in range(2):
                            nc.vector.bn_stats(
                                out=stats[:, sg, :],
                                in_=x_t[:, sg * 512:(sg + 1) * 512])
                        mv = stp.tile([128, 2], f32, tag="mv")
                        nc.vector.bn_aggr(out=mv, in_=stats)
                        nc.scalar.activation(
                            out=mv[:, 1:2], in_=mv[:, 1:2], func=AF.Sqrt,
                            bias=eps_t, scale=1.0)
                        nc.vector.reciprocal_approx_fast(
                            out=mv[:, 1:2], in_=mv[:, 1:2])
                        nc.vector.tensor_scalar(
                            out=x_t, in0=x_t, scalar1=mv[:, 0:1],
                            scalar2=mv[:, 1:2], op0=ALU.subtract,
                            op1=ALU.mult)
                        xts.append(x_t)

                    # transpose the group: 2 kc per big psum tile
                    xnT = [xtp.tile([128, 512], f32r, tag=f"x{kc}",
                                    name=f"xnT{kc}_{ig}") for kc in range(8)]
                    for kcp in range(4):
                        ps = bigp.tile([128, 1024], f32, tag="big",
                                       name=f"tp{ig}_{kcp}")
                        for half in range(2):
                            kc = 2 * kcp + half
                            for r in range(4):
                                nc.tensor.transpose(
                                    ps[:, half * 512 + r * 128:
                                       half * 512 + (r + 1) * 128],
                                    xts[r][:, kc * 128:(kc + 1) * 128],
                                    ident)
                            nc.scalar.copy(
                                out=xnT[kc],
                                in_=ps[:, half * 512:(half + 1) * 512])

                    if ig == 0:
                        nc.sync.dma_start(out=wq_sb, in_=wq[:, :, :])
                        nc.sync.dma_start(out=wk_sb, in_=wk[:, :, :])
                        nc.sync.dma_start(out=wv_sb, in_=wv[:, :, :])
                        nc.sync.dma_start(out=wo_sb, in_=wo[:, :, :])
                        for h in range(HPC):
                            nc.sync.dma_start(out=qhat[h][64:128, :],
                                              in_=qb[h])
                            nc.sync.dma_start(out=khat[h][64:128, :],
                                              in_=kb[:, :])

                    # q/k projections: emit as (proj, cc) pairs packed
                    # two-per-PSUM-tile; for the last i-group emit the
                    # heads-0/1 (cc=0) groups and v first so phase C can
                    # begin while heads-2/3 projections still run.
                    def emit_qk(pairs, tag_n):
                        ps = bigp.tile([128, 1024], f32, tag="big",
                                       name=f"qk{ig}_{tag_n}")
                        for half, (wt, dest, brow, cc) in enumerate(pairs):
                            psl = ps[:, half * 512:(half + 1) * 512]
                            for kc in range(8):
                                nc.tensor.matmul(
                                    psl,
                                    lhsT=wt[:, kc, cc * 128:(cc + 1) * 128],
                                    rhs=xnT[kc],
                                    start=(kc == 0),
                                    stop=(kc == 7 and brow is None))
                            if brow is not None:
                                nc.tensor.matmul(
                                    psl,
                                    lhsT=brow[:, cc * 128:(cc + 1) * 128],
                                    rhs=ones_row, start=False, stop=True)
                            nc.scalar.copy(
                                out=dest[2 * cc][0:64, isl], in_=psl[0:64, :])
                            nc.vector.tensor_copy(
                                out=dest[2 * cc + 1][0:64, isl],
                                in_=psl[64:128, :])

                    def emit_v():
                        for jp in range(2):
                            ps = bigp.tile([128, 1024], f32, tag="big",
                                           name=f"vv{ig}_{jp}")
                            for half in range(2):
                                jt = ig * 4 + jp * 2 + half
                                psl = ps[:, half * 512:half * 512 + 260]
                                for kc in range(8):
                                    nc.tensor.matmul(
                                        psl,
                                        lhsT=xnT[kc][:, (jp * 2 + half) * 128:
                                                     (jp * 2 + half + 1) * 128],
                                        rhs=wv_sb[:, kc, :],
                                        start=(kc == 0), stop=False)
                                nc.tensor.matmul(
                                    psl, lhsT=ones_row[:, 0:128],
                                    rhs=bv_row, start=False, stop=True)
                                nc.scalar.copy(out=v_sb[jt], in_=psl)

                    qp = (wq_sb, qhat, bq_row)
                    kp = (wk_sb, khat, None)
                    if ig < NI - 1:
                        emit_qk((qp + (0,), qp + (1,)), 0)
                        emit_qk((kp + (0,), kp + (1,)), 1)
                        emit_v()
                    else:
                        emit_qk((qp + (0,), kp + (0,)), 0)
                        emit_v()
                        emit_qk((qp + (1,), kp + (1,)), 1)

            # ------- Phase C+D: attention per head, out_proj interleaved -------
            # D row-tiles for i<1024 are emitted inside the last head's
            # second i-block so their PE work hides in exp-wait gaps.
            with tc.tile_pool(name="pt", bufs=6) as ptp, \
                 tc.tile_pool(name="rr", bufs=4) as rrp, \
                 tc.tile_pool(name="rb", bufs=4) as rbp, \
                 tc.tile_pool(name="yp", bufs=4) as yp:

                def emit_y(it):
                    tsl = slice(it * 128, (it + 1) * 128)
                    psy = [otpp.tile([128, 512], f32, tag=f"ot{nh}",
                                     name=f"y{it}_{nh}") for nh in range(2)]
                    y_t = yp.tile([128, D], f32, tag="y", name=f"yt{it}")
                    for nh in range(2):
                        for p in range(2):
                            nc.tensor.matmul(
                                psy[nh],
                                lhsT=ot_pair[p][:, tsl],
                                rhs=wo_sb[:, p, nh * 512:(nh + 1) * 512],
                                start=(p == 0), stop=(p == 1))
                    nc.vector.tensor_copy(out=y_t[:, 0:512], in_=psy[0])
                    nc.vector.tensor_copy(out=y_t[:, 512:1024], in_=psy[1])
                    nc.sync.dma_start(out=y[tsl, :], in_=y_t)

                for h in range(HPC):
                    vsl = slice(h * 65, (h + 1) * 65)
                    for ic2 in range(2):
                        ps_o = [otpp.tile([128, 512], f32, tag=f"ot{_o}",
                                          name=f"ot{_o}_{h}_{ic2}")
                                for _o in range(2)]
                        for jt in range(NT):
                            jsl = slice(jt * 128, (jt + 1) * 128)
                            st = bigp.tile([128, 1024], f32, tag="big",
                                           name=f"sc{h}_{ic2}_{jt}")
                            pt = ptp.tile([128, 1024], f32r, tag="pt")
                            for half in range(2):
                                i0 = ic2 * 1024 + half * 512
                                nc.tensor.matmul(
                                    st[:, half * 512:(half + 1) * 512],
                                    lhsT=khat[h][:, jsl],
                                    rhs=qhat[h][:, i0:i0 + 512],
                                    start=True, stop=True)
                            nc.scalar.activation(
                                out=pt, in_=st, func=AF.Exp,
                                bias=zero_col, scale=1.0)
                            for half in range(2):
                                nc.tensor.matmul(
                                    ps_o[half][0:65, :],
                                    lhsT=v_sb[jt][:, vsl],
                                    rhs=pt[:, half * 512:(half + 1) * 512],
                                    start=(jt == 0), stop=(jt == NT - 1))
                            if h == HPC - 1 and ic2 == 1 and jt % 2 == 1:
                                emit_y(jt // 2)  # i<1024 tiles: deps ready
                        hp, pair = h % 2, h // 2
                        for half in range(2):
                            i0 = ic2 * 1024 + half * 512
                            rr = rrp.tile([1, 512], f32, tag="rr")
                            # approx-recip mis-reads PSUM; bounce via SBUF
                            nc.vector.tensor_copy(
                                out=rr, in_=ps_o[half][64:65, :])
                            nc.vector.reciprocal_approx_fast(out=rr, in_=rr)
                            rb = rbp.tile([64, 512], f32, tag="rb")
                            nc.gpsimd.partition_broadcast(rb, rr)
                            nc.vector.tensor_mul(
                                out=ot_pair[pair][hp * 64:(hp + 1) * 64,
                                                  i0:i0 + 512],
                                in0=ps_o[half][0:64, :], in1=rb)

                for it in range(8, NT):
                    emit_y(it)

    nc.compile()
    return nc


def _host_prep(x, ln_g, ln_b, w_qkv, b_qkv, w_rel, w_out, b_out):
    """Per-core input dicts. LN affine is folded into w_qkv/b_qkv; the q-side
    softmax scale is folded into wq/bq; relative-position bias becomes the
    rank-64 (Qb, Kb) factor pair. Weights are pre-tiled to the on-chip
    layout so every DMA is a contiguous stream."""
    f32 = np.float32
    scale = HD ** -0.5
    W = (w_qkv.astype(f32) * ln_g.astype(f32)[:, None]).astype(f32)
    b_eff = (b_qkv.astype(f32) + ln_b.astype(f32) @ w_qkv.astype(f32)).astype(f32)

    # sinusoid tables (float64 for accuracy)
    omg = np.exp(np.arange(0, HD, 2, dtype=np.float64)
                 * (-math.log(10000.0) / HD))          # [32]
    ang = omg[:, None] * np.arange(T, dtype=np.float64)[None, :]  # [32, T]
    S, C = np.sin(ang), np.cos(ang)
    Kb = np.empty((HD, T), np.float64)
    Kb[0::2], Kb[1::2] = C, S
    Kb = Kb.astype(f32)
    ident = np.eye(128, dtype=f32)

    in_maps = []
    for c in range(NCORES):
        bi, hg = divmod(c, NCORES // B)
        hs = hg * HPC * HD  # 256-wide column block of this core's heads
        wqc = np.ascontiguousarray(W[:, hs:hs + 256] * scale)
        bqc = np.ascontiguousarray((b_eff[hs:hs + 256] * scale)[None, :])
        wkc = np.ascontiguousarray(W[:, D + hs:D + hs + 256])
        wvc = np.zeros((D, 260), f32)
        bvc = np.zeros((1, 260), f32)
        for h in range(HPC):
            csl = slice(2 * D + hs + h * HD, 2 * D + hs + (h + 1) * HD)
            wvc[:, h * 65:h * 65 + 64] = W[:, csl]
            bvc[0, h * 65:h * 65 + 64] = b_eff[csl]
            bvc[0, h * 65 + 64] = 1.0
        qbc = np.empty((HPC, HD, T), np.float64)
        for h in range(HPC):
            w2a = w_rel[0::2, hg * HPC + h].astype(np.float64)
            w2a1 = w_rel[1::2, hg * HPC + h].astype(np.float64)
            qbc[h, 0::2] = w2a[:, None] * S + w2a1[:, None] * C
            qbc[h, 1::2] = -w2a[:, None] * C + w2a1[:, None] * S
        in_maps.append({
            "x": np.ascontiguousarray(x[bi].astype(f32)),
            "wq": np.ascontiguousarray(
                wqc.reshape(8, 128, 256).transpose(1, 0, 2)),
            "wk": np.ascontiguousarray(
                wkc.reshape(8, 128, 256).transpose(1, 0, 2)),
            "wv": np.ascontiguousarray(
                wvc.reshape(8, 128, 260).transpose(1, 0, 2)),
            "bq": bqc, "bv": bvc,
            "qb": np.ascontiguousarray(qbc.astype(f32)),
            "kb": Kb,
            "wo": np.ascontiguousarray(
                w_out[hs:hs + 256, :].astype(f32).reshape(2, 128, D)
                .transpose(1, 0, 2)),
            "ident": ident, "ones": np.ones((1, 512), f32),
        })
    return in_maps


def kernel(x, ln_g, ln_b, w_qkv, b_qkv, w_rel, w_out, b_out):
    from concourse.bass_utils import run_bass_kernel_spmd

    x = np.asarray(x)
    nc = _build_nc()
    in_maps = _host_prep(x, np.asarray(ln_g), np.asarray(ln_b),
                         np.asarray(w_qkv), np.asarray(b_qkv),
                         np.asarray(w_rel), np.asarray(w_out),
                         np.asarray(b_out))
    res = run_bass_kernel_spmd(nc, in_maps, list(range(NCORES)))
    kernel._last_result = res
    cpb = NCORES // B
    y = np.empty((B, T, D), np.float32)
    for bi in range(B):
        acc = res.results[bi * cpb]["y"].astype(np.float32)
        for g in range(1, cpb):
            acc = acc + res.results[bi * cpb + g]["y"]
        y[bi] = acc + np.asarray(b_out, np.float32)[None, :]
    return y


# revision 26
# speedup vs baseline: 1.0308x; 1.0308x over previous
"""Trainium2 Bass kernel for MultiHeadSelfAttention with relative position bias.

Sharding: 8 cores = 2 batches x 4 head-groups (4 heads each).
Each core computes LN -> QKV (its heads) -> scores+softmax+AV -> partial
out_proj; host sums the 4 partials per batch (the all-reduce) and adds b_out.

Key tricks:
1. rel_bias[i-j] is exactly rank-64 in (i, j) (sinusoid angle-addition), so
   it folds into the scores matmul as 64 extra contraction rows:
       scores+bias = [kT; Kb]^T @ [qT_scaled; Qb]   (K = 128, full PE array)
   No Toeplitz gather, no bias-add pass.
2. Softmax denominators come free via an all-ones column appended to V
   (row 64 of the AV output = sum_j exp); no max-subtraction is needed
   (|scores| < ~20, exp is safe in fp32) and the 1/sum normalization is
   deferred across the linear AV matmul into the OT evacuation.
3. float32r matmuls run at full PE rate (1 cycle/row at N>=512).
"""

import functools
import math
import sys

import numpy as np

for _p in ("/opt/trn_rl_repo", "/root/.axon_site/_ro/trn_rl_repo"):
    if _p not in sys.path:
        sys.path.insert(0, _p)

B, T, D, H, HD = 2, 2048, 1024, 16, 64
NCORES = 8
HPC = 4  # heads per core
NT = T // 128  # 16 row tiles
NI = T // 512  # 4 i-chunks (also LN i-groups of 4 row tiles)
F32R = True  # float32r (full-rate) matmuls; False = exact fp32 at 1/4 rate


@functools.lru_cache(maxsize=1)
def _build_nc():
    import concourse.mybir as mybir
    import concourse.tile as tile
    from concourse import bacc

    dt = mybir.dt
    f32 = dt.float32
    f32r = dt.float32r if F32R else f32
    AF = mybir.ActivationFunctionType
    ALU = mybir.AluOpType

    nc = bacc.Bacc("TRN2", target_bir_lowering=False, debug=False,
                   num_devices=NCORES)
    x = nc.declare_dram_parameter("x", [T, D], f32, isOutput=False)
    wq = nc.declare_dram_parameter("wq", [128, 8, 256], f32r, isOutput=False)
    wk = nc.declare_dram_parameter("wk", [128, 8, 256], f32r, isOutput=False)
    wv = nc.declare_dram_parameter("wv", [128, 8, 260], f32r, isOutput=False)
    bq = nc.declare_dram_parameter("bq", [1, 256], f32r, isOutput=False)
    bv = nc.declare_dram_parameter("bv", [1, 260], f32r, isOutput=False)
    qb = nc.declare_dram_parameter("qb", [HPC, 64, T], f32r, isOutput=False)
    kb = nc.declare_dram_parameter("kb", [64, T], f32r, isOutput=False)
    wo = nc.declare_dram_parameter("wo", [128, 2, D], f32r, isOutput=False)
    idin = nc.declare_dram_parameter("ident", [128, 128], f32, isOutput=False)
    onesd = nc.declare_dram_parameter("ones", [1, 512], f32r, isOutput=False)
    y = nc.declare_dram_parameter("y", [T, D], f32, isOutput=True)

    with tile.TileContext(nc) as tc:
        with tc.tile_pool(name="persist", bufs=1) as pp, \
             tc.tile_pool(name="big_ps", bufs=2, space="PSUM") as bigp, \
             tc.tile_pool(name="ot_ps", bufs=2, space="PSUM") as otpp:
            zero_col = pp.tile([128, 1], f32, tag="zero")
            nc.vector.memset(zero_col, 0.0)
            qhat = [pp.tile([128, T], f32r, tag=f"qhat{h}", name=f"qhat{h}")
                    for h in range(HPC)]
            khat = [pp.tile([128, T], f32r, tag=f"khat{h}", name=f"khat{h}")
                    for h in range(HPC)]
            v_sb = [pp.tile([128, 260], f32r, tag=f"v{jt}", name=f"v{jt}")
                    for jt in range(NT)]
            ot_pair = [pp.tile([128, T], f32r, tag=f"ot{p}", name=f"otp{p}")
                       for p in range(2)]
            wq_sb = pp.tile([128, 8, 256], f32r, tag="wq")
            wk_sb = pp.tile([128, 8, 256], f32r, tag="wk")
            wv_sb = pp.tile([128, 8, 260], f32r, tag="wv")
            wo_sb = pp.tile([128, 2, D], f32r, tag="wo")

            # ------------- Phase A+B interleaved per i-group -------------
            with tc.tile_pool(name="ab_sb", bufs=1) as abp, \
                 tc.tile_pool(name="ln", bufs=8) as lnp, \
                 tc.tile_pool(name="stats", bufs=4) as stp, \
                 tc.tile_pool(name="xnT", bufs=2) as xtp:
                ident = abp.tile([128, 128], f32, tag="ident")
                nc.sync.dma_start(out=ident, in_=idin[:, :])
                eps_t = abp.tile([128, 1], f32, tag="eps")
                nc.vector.memset(eps_t, 1e-5)
                ones_row = abp.tile([1, 512], f32r, tag="ones")
                nc.sync.dma_start(out=ones_row, in_=onesd[:, :])
                bq_row = abp.tile([1, 256], f32r, tag="bqr")
                nc.sync.dma_start(out=bq_row, in_=bq[:, :])
                bv_row = abp.tile([1, 260], f32r, tag="bvr")
                nc.sync.dma_start(out=bv_row, in_=bv[:, :])

                for ig in range(NI):
                    isl = slice(ig * 512, (ig + 1) * 512)
                    xts = []
                    for r in range(4):
                        it = ig * 4 + r
                        x_t = lnp.tile([128, D], f32, tag="x", name=f"x{it}")
                        nc.sync.dma_start(
                            out=x_t, in_=x[it * 128:(it + 1) * 128, :])
                        stats = stp.tile([128, 2, 6], f32, tag="st")
                        for sg in range(2):
                            nc.vector.bn_stats(
                                out=stats[:, sg, :],
                                in_=x_t[:, sg * 512:(sg + 1) * 512])
                        mv = stp.tile([128, 2], f32, tag="mv")
                        nc.vector.bn_aggr(out=mv, in_=stats)
                        nc.scalar.activation(
                            out=mv[:, 1:2], in_=mv[:, 1:2], func=AF.Sqrt,
                            bias=eps_t, scale=1.0)
                        nc.vector.reciprocal_approx_fast(
                            out=mv[:, 1:2], in_=mv[:, 1:2])
                        nc.vector.tensor_scalar(
                            out=x_t, in0=x_t, scalar1=mv[:, 0:1],
                            scalar2=mv[:, 1:2], op0=ALU.subtract,
                            op1=ALU.mult)
                        xts.append(x_t)

                    # transpose the group: 2 kc per big psum tile
                    xnT = [xtp.tile([128, 512], f32r, tag=f"x{kc}",
                                    name=f"xnT{kc}_{ig}") for kc in range(8)]
                    for kcp in range(4):
                        ps = bigp.tile([128, 1024], f32, tag="big",
                                       name=f"tp{ig}_{kcp}")
                        for half in range(2):
                            kc = 2 * kcp + half
                            for r in range(4):
                                nc.tensor.transpose(
                                    ps[:, half * 512 + r * 128:
                                       half * 512 + (r + 1) * 128],
                                    xts[r][:, kc * 128:(kc + 1) * 128],
                                    ident)
                            nc.scalar.copy(
                                out=xnT[kc],
                                in_=ps[:, half * 512:(half + 1) * 512])

                    if ig == 0:
                        nc.sync.dma_start(out=wq_sb, in_=wq[:, :, :])
                        nc.sync.dma_start(out=wk_sb, in_=wk[:, :, :])
                        nc.sync.dma_start(out=wv_sb, in_=wv[:, :, :])
                        nc.sync.dma_start(out=wo_sb, in_=wo[:, :, :])

                    # q/k projections for this i-chunk (both cc in one tile)
                    for wt, dest, brow in ((wq_sb, qhat, bq_row),
                                           (wk_sb, khat, None)):
                        ps = bigp.tile([128, 1024], f32, tag="big",
                                       name=f"qk{ig}")
                        for cc in range(2):
                            psl = ps[:, cc * 512:(cc + 1) * 512]
                            for kc in range(8):
                                nc.tensor.matmul(
                                    psl,
                                    lhsT=wt[:, kc, cc * 128:(cc + 1) * 128],
                                    rhs=xnT[kc],
                                    start=(kc == 0),
                                    stop=(kc == 7 and brow is None))
                            if brow is not None:
                                nc.tensor.matmul(
                                    psl,
                                    lhsT=brow[:, cc * 128:(cc + 1) * 128],
                                    rhs=ones_row, start=False, stop=True)
                            nc.scalar.copy(
                                out=dest[2 * cc][0:64, isl], in_=psl[0:64, :])
                            nc.vector.tensor_copy(
                                out=dest[2 * cc + 1][0:64, isl],
                                in_=psl[64:128, :])

                    # v projections for the 4 j-tiles of this group
                    for jp in range(2):
                        ps = bigp.tile([128, 1024], f32, tag="big",
                                       name=f"vv{ig}_{jp}")
                        for half in range(2):
                            jt = ig * 4 + jp * 2 + half
                            psl = ps[:, half * 512:half * 512 + 260]
                            for kc in range(8):
                                nc.tensor.matmul(
                                    psl,
                                    lhsT=xnT[kc][:, (jp * 2 + half) * 128:
                                                 (jp * 2 + half + 1) * 128],
                                    rhs=wv_sb[:, kc, :],
                                    start=(kc == 0), stop=False)
                            nc.tensor.matmul(
                                psl, lhsT=ones_row[:, 0:128],
                                rhs=bv_row, start=False, stop=True)
                            nc.scalar.copy(out=v_sb[jt], in_=psl)

                # bias factor tables (needed first by phase C)
                for h in range(HPC):
                    nc.sync.dma_start(out=qhat[h][64:128, :], in_=qb[h])
                    nc.sync.dma_start(out=khat[h][64:128, :], in_=kb[:, :])

            # ------- Phase C+D: attention per head, out_proj interleaved -------
            # D row-tiles for i<1024 are emitted inside the last head's
            # second i-block so their PE work hides in exp-wait gaps.
            with tc.tile_pool(name="pt", bufs=6) as ptp, \
                 tc.tile_pool(name="rr", bufs=4) as rrp, \
                 tc.tile_pool(name="rb", bufs=4) as rbp, \
                 tc.tile_pool(name="yp", bufs=4) as yp:

                def emit_y(it):
                    tsl = slice(it * 128, (it + 1) * 128)
                    psy = [otpp.tile([128, 512], f32, tag=f"ot{nh}",
                                     name=f"y{it}_{nh}") for nh in range(2)]
                    y_t = yp.tile([128, D], f32, tag="y", name=f"yt{it}")
                    for nh in range(2):
                        for p in range(2):
                            nc.tensor.matmul(
                                psy[nh],
                                lhsT=ot_pair[p][:, tsl],
                                rhs=wo_sb[:, p, nh * 512:(nh + 1) * 512],
                                start=(p == 0), stop=(p == 1))
                    nc.vector.tensor_copy(out=y_t[:, 0:512], in_=psy[0])
                    nc.vector.tensor_copy(out=y_t[:, 512:1024], in_=psy[1])
                    nc.sync.dma_start(out=y[tsl, :], in_=y_t)

                for h in range(HPC):
                    vsl = slice(h * 65, (h + 1) * 65)
                    for ic2 in range(2):
                        ps_o = [otpp.tile([128, 512], f32, tag=f"ot{_o}",
                                          name=f"ot{_o}_{h}_{ic2}")
                                for _o in range(2)]
                        for jt in range(NT):
                            jsl = slice(jt * 128, (jt + 1) * 128)
                            st = bigp.tile([128, 1024], f32, tag="big",
                                           name=f"sc{h}_{ic2}_{jt}")
                            pt = ptp.tile([128, 1024], f32r, tag="pt")
                            for half in range(2):
                                i0 = ic2 * 1024 + half * 512
                                nc.tensor.matmul(
                                    st[:, half * 512:(half + 1) * 512],
                                    lhsT=khat[h][:, jsl],
                                    rhs=qhat[h][:, i0:i0 + 512],
                                    start=True, stop=True)
                            nc.scalar.activation(
                                out=pt, in_=st, func=AF.Exp,
                                bias=zero_col, scale=1.0)
                            for half in range(2):
                                nc.tensor.matmul(
                                    ps_o[half][0:65, :],
                                    lhsT=v_sb[jt][:, vsl],
                                    rhs=pt[:, half * 512:(half + 1) * 512],
                                    start=(jt == 0), stop=(jt == NT - 1))
                            if h == HPC - 1 and ic2 == 1 and jt % 2 == 1:
                                emit_y(jt // 2)  # i<1024 tiles: deps ready
                        hp, pair = h % 2, h // 2
                        for half in range(2):
                            i0 = ic2 * 1024 + half * 512
                            rr = rrp.tile([1, 512], f32, tag="rr")
                            # approx-recip mis-reads PSUM; bounce via SBUF
                            nc.vector.tensor_copy(
                                out=rr, in_=ps_o[half][64:65, :])
                            nc.vector.reciprocal_approx_fast(out=rr, in_=rr)
                            rb = rbp.tile([64, 512], f32, tag="rb")
                            nc.gpsimd.partition_broadcast(rb, rr)
                            nc.vector.tensor_mul(
                                out=ot_pair[pair][hp * 64:(hp + 1) * 64,
                                                  i0:i0 + 512],
                                in0=ps_o[half][0:64, :], in1=rb)

                for it in range(8, NT):
                    emit_y(it)

    nc.compile()
    return nc


def _host_prep(x, ln_g, ln_b, w_qkv, b_qkv, w_rel, w_out, b_out):
    """Per-core input dicts. LN affine is folded into w_qkv/b_qkv; the q-side
    softmax scale is folded into wq/bq; relative-position bias becomes the
    rank-64 (Qb, Kb) factor pair. Weights are pre-tiled to the on-chip
    layout so every DMA is a contiguous stream."""
    f32 = np.float32
    scale = HD ** -0.5
    W = (w_qkv.astype(f32) * ln_g.astype(f32)[:, None]).astype(f32)
    b_eff = (b_qkv.astype(f32) + ln_b.astype(f32) @ w_qkv.astype(f32)).astype(f32)

    # sinusoid tables (float64 for accuracy)
    omg = np.exp(np.arange(0, HD, 2, dtype=np.float64)
                 * (-math.log(10000.0) / HD))          # [32]
    ang = omg[:, None] * np.arange(T, dtype=np.float64)[None, :]  # [32, T]
    S, C = np.sin(ang), np.cos(ang)
    Kb = np.empty((HD, T), np.float64)
    Kb[0::2], Kb[1::2] = C, S
    Kb = Kb.astype(f32)
    ident = np.eye(128, dtype=f32)

    in_maps = []
    for c in range(NCORES):
        bi, hg = divmod(c, NCORES // B)
        hs = hg * HPC * HD  # 256-wide column block of this core's heads
        wqc = np.ascontiguousarray(W[:, hs:hs + 256] * scale)
        bqc = np.ascontiguousarray((b_eff[hs:hs + 256] * scale)[None, :])
        wkc = np.ascontiguousarray(W[:, D + hs:D + hs + 256])
        wvc = np.zeros((D, 260), f32)
        bvc = np.zeros((1, 260), f32)
        for h in range(HPC):
            csl = slice(2 * D + hs + h * HD, 2 * D + hs + (h + 1) * HD)
            wvc[:, h * 65:h * 65 + 64] = W[:, csl]
            bvc[0, h * 65:h * 65 + 64] = b_eff[csl]
            bvc[0, h * 65 + 64] = 1.0
        qbc = np.empty((HPC, HD, T), np.float64)
        for h in range(HPC):
            w2a = w_rel[0::2, hg * HPC + h].astype(np.float64)
            w2a1 = w_rel[1::2, hg * HPC + h].astype(np.float64)
            qbc[h, 0::2] = w2a[:, None] * S + w2a1[:, None] * C
            qbc[h, 1::2] = -w2a[:, None] * C + w2a1[:, None] * S
        in_maps.append({
            "x": np.ascontiguousarray(x[bi].astype(f32)),
            "wq": np.ascontiguousarray(
                wqc.reshape(8, 128, 256).transpose(1, 0, 2)),
            "wk": np.ascontiguousarray(
                wkc.reshape(8, 128, 256).transpose(1, 0, 2)),
            "wv": np.ascontiguousarray(
                wvc.reshape(8, 128, 260).transpose(1, 0, 2)),
            "bq": bqc, "bv": bvc,
            "qb": np.ascontiguousarray(qbc.astype(f32)),
            "kb": Kb,
            "wo": np.ascontiguousarray(
                w_out[hs:hs + 256, :].astype(f32).reshape(2, 128, D)
                .transpose(1, 0, 2)),
            "ident": ident, "ones": np.ones((1, 512), f32),
        })
    return in_maps


def kernel(x, ln_g, ln_b, w_qkv, b_qkv, w_rel, w_out, b_out):
    from concourse.bass_utils import run_bass_kernel_spmd

    x = np.asarray(x)
    nc = _build_nc()
    in_maps = _host_prep(x, np.asarray(ln_g), np.asarray(ln_b),
                         np.asarray(w_qkv), np.asarray(b_qkv),
                         np.asarray(w_rel), np.asarray(w_out),
                         np.asarray(b_out))
    res = run_bass_kernel_spmd(nc, in_maps, list(range(NCORES)))
    kernel._last_result = res
    cpb = NCORES // B
    y = np.empty((B, T, D), np.float32)
    for bi in range(B):
        acc = res.results[bi * cpb]["y"].astype(np.float32)
        for g in range(1, cpb):
            acc = acc + res.results[bi * cpb + g]["y"]
        y[bi] = acc + np.asarray(b_out, np.float32)[None, :]
    return y
